# revision 1
# baseline (speedup 1.0000x reference)
"""GatedCRFLoss Trainium2 kernel v2: 8-core SPMD over (B,H) row stripes.

Per-pair pipeline (offsets (dx,dy) & (-dx,-dy) share kernel values):
  X = su*I(u) - sv*I(v)          one TT add (v-side tile pre-negated, planar)
  g_c = DErf(X_c) = (2/sqrt(pi)) exp(-0.5 (dI_c/sigma_c)^2)   one Act op, 4 planes
  GP = g0*g1*g2 ;  K = kappa'*GP + g3   (kappa' = kappa*pi/4; host multiplies
                                         numerators by sqrt(pi)/2)
  P = sum_c pu_c * pv_c ;  AN = (P-1)*K
  ZS1 = sum -AN*dst_v -> col k ; ZS2 = sum -AN*dst_u*mask -> col NPAIR+k
dx values are processed in groups of 2 so most DVE/Act ops batch two dx
at once (free dims [2, planes, W]).
"""
import sys

sys.path.insert(0, "/opt/trn_rl_repo")

import math
import numpy as np
import ml_dtypes

SPAN = 11
B, C, H, W = 4, 3, 256, 512
NCORES = 8
RPC = 128
HALO = SPAN
SIG_RGB = 0.1
SIG_XY = 6.0
SIG_DEPTH = 0.2
PRESCALE_RGB = 1.0 / (SIG_RGB * math.sqrt(2.0))
PRESCALE_DEP = 1.0 / (SIG_DEPTH * math.sqrt(2.0))
HOST_NUM_SCALE = math.sqrt(math.pi) / 2.0  # multiplies device numerators

DYS = list(range(-SPAN, 0)) + list(range(1, SPAN + 1))
PAIRS = [(dx, dy) for dx in range(1, SPAN + 1) for dy in DYS]
NPAIR = len(PAIRS)  # 242
COL_CE_LDS = 2 * NPAIR      # 484: sum(L_ce * dst)
COL_CE_L = 2 * NPAIR + 1    # 485: sum(L_ce)
ACC_W = 512

GROUPS = [(1, 2), (3, 4), (5, 6), (7, 8), (9, 10), (11,)]

# engine assignment knobs
GG1_ENG = "vector"   # g0*g1
M3_ENG = "vector"    # prob products (gpsimd crashes on batched 3-dim APs)
P1_ENG = "gpsimd"    # m0+m1 (2-free-dim: pool-safe)
GG_ENG = "vector"    # GG1*g2 (pool chain serializes)
ZS1_ENG = "vector"   # accum vs dst_v (Pool lacks STT)
ZS2_ENG = "vector"   # accum vs dst_u*mask
K_ENG = "vector"     # per-dx STT
X8_DEPTH_PE = True  # depth-plane diff on the tensor engine (PSUM + split DErf)
_CACHE = {}


def _kappa(dx, dy):
    return math.exp(-0.5 * (dx * dx + dy * dy) / (SIG_XY * SIG_XY)) * math.pi / 4.0


def _build():
    import concourse.bass as bass  # noqa: F401
    import concourse.tile as tile
    from concourse import bacc, mybir

    BF = mybir.dt.bfloat16
    F32 = mybir.dt.float32
    Alu = mybir.AluOpType
    Act = mybir.ActivationFunctionType

    nc = bacc.Bacc("TRN2", target_bir_lowering=False, debug=False,
                   num_devices=NCORES)

    imgu_d = nc.dram_tensor("imgu", [RPC, 4 * W], BF, kind="ExternalInput").ap()
    imgv_d = nc.dram_tensor("imgv", [RPC + HALO, 4 * W], BF, kind="ExternalInput").ap()
    lg_d = nc.dram_tensor("lg", [RPC + HALO, 3 * W], F32, kind="ExternalInput").ap()
    ds_d = nc.dram_tensor("ds", [RPC + HALO, W], F32, kind="ExternalInput").ap()
    tgt_d = nc.dram_tensor("tgt", [RPC, W], F32, kind="ExternalInput").ap()
    dsm_d = nc.dram_tensor("dsmall", [RPC, SPAN * W], BF, kind="ExternalInput").ap()
    out_d = nc.dram_tensor("out", [RPC, 2 * ACC_W], F32, kind="ExternalOutput").ap()
    prb_d = nc.dram_tensor("prb_scratch", [RPC + HALO, 4 * W], BF).ap()

    imgv3 = imgv_d.rearrange("r (c w) -> r c w", w=W)
    lg3 = lg_d.rearrange("r (c w) -> r c w", w=W)
    prb3 = prb_d.rearrange("r (c w) -> r c w", w=W)
    dsm3 = dsm_d.rearrange("r (c w) -> r c w", w=W)

    eng = {"gpsimd": nc.gpsimd, "vector": nc.vector}
    eye_d = nc.dram_tensor("eye", [RPC, RPC], BF, kind="ExternalInput").ap()

    with tile.TileContext(nc) as tc:
        from contextlib import ExitStack
        with ExitStack() as ctx:
            cp = ctx.enter_context(tc.tile_pool(name="const", bufs=1))
            vp = ctx.enter_context(tc.tile_pool(name="vshift", bufs=2))
            tp = ctx.enter_context(tc.tile_pool(name="tmp", bufs=2))
            pp = ctx.enter_context(
                tc.tile_pool(name="ps", bufs=2, space=bass.MemorySpace.PSUM))
            EYE = cp.tile([RPC, RPC], BF, tag="EYE")
            nc.sync.dma_start(EYE[:, :], eye_d[:, :])

            IMU2 = cp.tile([RPC, 2, 4, W], BF, tag="IMU2")
            PRB2 = cp.tile([RPC, 2, 4, W], BF, tag="PRB2")
            PRBH = cp.tile([HALO, 4, W], BF, tag="PRBH")
            LG = cp.tile([RPC, 3, W], F32, tag="LG")
            LGH = cp.tile([HALO, 3, W], F32, tag="LGH")
            TGT = cp.tile([RPC, W], F32, tag="TGT")
            DSF = cp.tile([RPC, W], F32, tag="DSF")
            DSFH = cp.tile([HALO, W], F32, tag="DSFH")
            DSM = cp.tile([RPC, SPAN, W], BF, tag="DSM")
            ACCV = cp.tile([RPC, ACC_W], F32, tag="ACCV")
            ACCP = cp.tile([RPC, ACC_W], F32, tag="ACCP")

            nc.sync.dma_start(IMU2[:, 0, :, :],
                              imgu_d.rearrange("r (c w) -> r c w", w=W)[:, :, :])
            nc.sync.dma_start(IMU2[:, 1, :, :],
                              imgu_d.rearrange("r (c w) -> r c w", w=W)[:, :, :])
            nc.sync.dma_start(LG[:, :, :], lg3[HALO:, :, :])
            nc.sync.dma_start(LGH[:, :, :], lg3[0:HALO, :, :])
            nc.sync.dma_start(TGT[:, :], tgt_d[:, :])
            nc.sync.dma_start(DSF[:, :], ds_d[HALO:, :])
            nc.sync.dma_start(DSFH[:, :], ds_d[0:HALO, :])
            nc.sync.dma_start(DSM[:, :, :], dsm3[:, :, :])

            nc.gpsimd.memset(ACCP[:, :], 0.0)
            nc.vector.memset(ACCV[:, :], 0.0)

            # ---- softmax (main stripe) into planar PRB2[:,0] and PRB2[:,1] ----
            EXL = cp.tile([RPC, 3, W], F32, tag="EXL")
            SS = cp.tile([RPC, W], F32, tag="SS")
            RR = cp.tile([RPC, W], F32, tag="RR")
            nc.scalar.activation(EXL[:, :, :], LG[:, :, :], Act.Exp)
            nc.vector.tensor_add(SS[:, :], EXL[:, 0, :], EXL[:, 1, :])
            nc.vector.tensor_tensor(SS[:, :], SS[:, :], EXL[:, 2, :], Alu.add)
            nc.vector.reciprocal(RR[:, :], SS[:, :])
            for c in range(3):
                nc.vector.tensor_mul(PRB2[:, 0, c, :], EXL[:, c, :], RR[:, :])
            nc.vector.tensor_copy(PRB2[:, 0, 3, :], DSF[:, :])
            nc.vector.tensor_copy(PRB2[:, 1, :, :], PRB2[:, 0, :, :])

            # ---- softmax (halo rows) ----
            EXLH = cp.tile([HALO, 3, W], F32, tag="EXLH")
            SSH = cp.tile([HALO, W], F32, tag="SSH")
            RRH = cp.tile([HALO, W], F32, tag="RRH")
            nc.scalar.activation(EXLH[:, :, :], LGH[:, :, :], Act.Exp)
            nc.vector.tensor_add(SSH[:, :], EXLH[:, 0, :], EXLH[:, 1, :])
            nc.vector.tensor_tensor(SSH[:, :], SSH[:, :], EXLH[:, 2, :], Alu.add)
            nc.vector.reciprocal(RRH[:, :], SSH[:, :])
            for c in range(3):
                nc.vector.tensor_mul(PRBH[:, c, :], EXLH[:, c, :], RRH[:, :])
            nc.vector.tensor_copy(PRBH[:, 3, :], DSFH[:, :])

            nc.sync.dma_start(prb3[HALO:, :, :], PRB2[:, 0, :, :])
            nc.sync.dma_start(prb3[0:HALO, :, :], PRBH[:, :, :])

            # ---- cross entropy partials (fp32, on DVE) ----
            LS = cp.tile([RPC, W], F32, tag="LS")
            M1 = cp.tile([RPC, W], F32, tag="M1")
            M2 = cp.tile([RPC, W], F32, tag="M2")
            D10 = cp.tile([RPC, W], F32, tag="D10")
            D21 = cp.tile([RPC, W], F32, tag="D21")
            T1 = cp.tile([RPC, W], F32, tag="T1")
            LT = cp.tile([RPC, W], F32, tag="LT")
            LCE = cp.tile([RPC, W], F32, tag="LCE")
            CES = cp.tile([RPC, W], F32, tag="CES")
            nc.scalar.activation(LS[:, :], SS[:, :], Act.Ln)
            nc.vector.tensor_scalar(M1[:, :], TGT[:, :], 0.5, None, Alu.is_ge)
            nc.vector.tensor_scalar(M2[:, :], TGT[:, :], 1.5, None, Alu.is_ge)
            nc.vector.tensor_sub(D10[:, :], LG[:, 1, :], LG[:, 0, :])
            nc.vector.tensor_sub(D21[:, :], LG[:, 2, :], LG[:, 1, :])
            nc.vector.tensor_mul(T1[:, :], M1[:, :], D10[:, :])
            nc.vector.tensor_add(LT[:, :], LG[:, 0, :], T1[:, :])
            nc.vector.tensor_mul(T1[:, :], M2[:, :], D21[:, :])
            nc.vector.tensor_tensor(LT[:, :], LT[:, :], T1[:, :], Alu.add)
            nc.vector.tensor_sub(LCE[:, :], LS[:, :], LT[:, :])
            nc.vector.scalar_tensor_tensor(
                CES[:, :], LCE[:, :], 1.0, DSF[:, :],
                Alu.mult, Alu.mult,
                accum_out=ACCV[:, COL_CE_LDS:COL_CE_LDS + 1])
            nc.vector.tensor_reduce(
                ACCV[:, COL_CE_L:COL_CE_L + 1], LCE[:, :],
                mybir.AxisListType.X, Alu.add)

            # ---- pair loop: dx groups of 2 ----
            for grp in GROUPS:
                ng = len(grp)
                IMV = vp.tile([RPC, 2, 4, W], BF, tag="IMV")
                PVS = vp.tile([RPC, 2, 4, W], BF, tag="PVS")
                for i, dx in enumerate(grp):
                    nc.sync.dma_start(IMV[:, i, :, :],
                                      imgv3[HALO - dx:HALO - dx + RPC, :, :])
                    nc.sync.dma_start(PVS[:, i, :, :],
                                      prb3[HALO - dx:HALO - dx + RPC, :, :])

                for dy in DYS:
                    ady = abs(dy)
                    FD = W - ady
                    us, vs = (ady, 0) if dy > 0 else (0, ady)

                    G4 = tp.tile([RPC, 2, 4, W], BF, tag="G4")
                    if X8_DEPTH_PE:
                        XP = pp.tile([RPC, 2, W], F32, tag="XP")
                        for i in range(ng):
                            nc.tensor.matmul(
                                XP[:, i, 0:FD], EYE[:, :],
                                IMU2[:, i, 3, us:us + FD],
                                start=True, stop=False)
                            nc.tensor.matmul(
                                XP[:, i, 0:FD], EYE[:, :],
                                IMV[:, i, 3, vs:vs + FD],
                                start=False, stop=True)
                        X8 = tp.tile([RPC, 2, 3, W], BF, tag="X8")
                        nc.vector.tensor_tensor(
                            X8[:, 0:ng, :, 0:FD], IMU2[:, 0:ng, 0:3, us:us + FD],
                            IMV[:, 0:ng, 0:3, vs:vs + FD], Alu.add)
                        nc.scalar.activation(G4[:, 0:ng, 0:3, 0:FD],
                                             X8[:, 0:ng, :, 0:FD],
                                             Act.Derivative_Erf)
                        nc.scalar.activation(G4[:, 0:ng, 3, 0:FD],
                                             XP[:, 0:ng, 0:FD],
                                             Act.Derivative_Erf)
                    else:
                        X8 = tp.tile([RPC, 2, 4, W], BF, tag="X8")
                        nc.vector.tensor_tensor(
                            X8[:, 0:ng, :, 0:FD], IMU2[:, 0:ng, :, us:us + FD],
                            IMV[:, 0:ng, :, vs:vs + FD], Alu.add)
                        nc.scalar.activation(G4[:, 0:ng, :, 0:FD],
                                             X8[:, 0:ng, :, 0:FD],
                                             Act.Derivative_Erf)
                    GG1 = tp.tile([RPC, 2, W], BF, tag="GG1")
                    eng[GG1_ENG].tensor_tensor(
                        GG1[:, 0:ng, 0:FD], G4[:, 0:ng, 0, 0:FD],
                        G4[:, 0:ng, 1, 0:FD], Alu.mult)
                    GG = tp.tile([RPC, 2, W], BF, tag="GG")
                    eng[GG_ENG].tensor_tensor(
                        GG[:, 0:ng, 0:FD], GG1[:, 0:ng, 0:FD],
                        G4[:, 0:ng, 2, 0:FD], Alu.mult)
                    KT = tp.tile([RPC, 2, W], BF, tag="KT")
                    for i, dx in enumerate(grp):
                        nc.vector.tensor_scalar(
                            KT[:, i, 0:FD], GG[:, i, 0:FD], _kappa(dx, dy),
                            None, Alu.mult)
                    K = tp.tile([RPC, 2, W], BF, tag="K")
                    nc.vector.tensor_tensor(
                        K[:, 0:ng, 0:FD], KT[:, 0:ng, 0:FD],
                        G4[:, 0:ng, 3, 0:FD], Alu.add)
                    M3 = tp.tile([RPC, 2, 3, W], BF, tag="M3")
                    eng[M3_ENG].tensor_tensor(
                        M3[:, 0:ng, :, 0:FD], PRB2[:, 0:ng, 0:3, us:us + FD],
                        PVS[:, 0:ng, 0:3, vs:vs + FD], Alu.mult)
                    P1 = tp.tile([RPC, 2, W], BF, tag="P1")
                    eng[P1_ENG].tensor_tensor(
                        P1[:, 0:ng, 0:FD], M3[:, 0:ng, 0, 0:FD],
                        M3[:, 0:ng, 1, 0:FD], Alu.add)
                    P = tp.tile([RPC, 2, W], BF, tag="P")
                    nc.vector.tensor_tensor(
                        P[:, 0:ng, 0:FD], P1[:, 0:ng, 0:FD],
                        M3[:, 0:ng, 2, 0:FD], Alu.add)
                    ANt = tp.tile([RPC, 2, W], BF, tag="ANt")
                    nc.vector.tensor_scalar(
                        ANt[:, 0:ng, 0:FD], P[:, 0:ng, 0:FD], 1.0, None,
                        Alu.subtract)
                    AN = tp.tile([RPC, 2, W], BF, tag="AN")
                    nc.vector.tensor_tensor(
                        AN[:, 0:ng, 0:FD], ANt[:, 0:ng, 0:FD],
                        K[:, 0:ng, 0:FD], Alu.mult)
                    ZS1 = tp.tile([RPC, 2, W], BF, tag="ZS1")
                    ZS2 = tp.tile([RPC, 2, W], BF, tag="ZS2")
                    for i, dx in enumerate(grp):
                        k = PAIRS.index((dx, dy))
                        nc.vector.scalar_tensor_tensor(
                            ZS1[:, i, 0:FD], AN[:, i, 0:FD], -1.0,
                            PVS[:, i, 3, vs:vs + FD], Alu.mult, Alu.mult,
                            accum_out=ACCV[:, k:k + 1])
                        nc.vector.scalar_tensor_tensor(
                            ZS2[:, i, 0:FD], AN[:, i, 0:FD], -1.0,
                            DSM[:, dx - 1, us:us + FD], Alu.mult, Alu.mult,
                            accum_out=ACCV[:, NPAIR + k:NPAIR + k + 1])

            nc.sync.dma_start(out_d[:, 0:ACC_W], ACCV[:, :])
            nc.sync.dma_start(out_d[:, ACC_W:2 * ACC_W], ACCP[:, :])

    nc.compile()
    return nc


def _get_nc():
    if "nc" not in _CACHE:
        _CACHE["nc"] = _build()
    return _CACHE["nc"]


def _make_inputs(logit, target, image, depth, destination_map):
    """Build the 8 per-core input dicts (planar, prescaled, v-side negated)."""
    bf = ml_dtypes.bfloat16
    in_maps = []
    scales = np.array([PRESCALE_RGB] * 3 + [PRESCALE_DEP], np.float32)
    for cidx in range(NCORES):
        b = cidx // 2
        r0 = RPC * (cidx % 2)
        rows = np.arange(r0 - HALO, r0 + RPC)
        valid = rows >= 0
        rv = np.clip(rows, 0, H - 1)

        def stripe(x2d, zero_invalid=True):
            s = x2d[rv].astype(np.float32)
            if zero_invalid:
                s[~valid] = 0.0
            return s

        img4 = np.zeros((RPC + HALO, 4, W), np.float32)
        for c in range(3):
            img4[:, c, :] = stripe(np.asarray(image[b, c]))
        img4[:, 3, :] = stripe(np.asarray(depth[b, 0]))
        img4 *= scales[None, :, None]
        imgu = img4[HALO:]
        imgv = -img4
        lg = np.zeros((RPC + HALO, 3, W), np.float32)
        for c in range(3):
            lg[:, c, :] = stripe(np.asarray(logit[b, c]))
        ds = stripe(np.asarray(destination_map[b, 0]))
        tgt = np.asarray(target[b, r0:r0 + RPC]).astype(np.float32)
        # dst_u * (v-row valid) per dx
        dsmall = np.zeros((RPC, SPAN, W), np.float32)
        for dx in range(1, SPAN + 1):
            m = ds[HALO:].copy()
            lim = dx - r0
            if lim > 0:
                m[:lim] = 0.0
            dsmall[:, dx - 1, :] = m
        in_maps.append({
            "eye": np.eye(RPC, dtype=np.float32).astype(bf),
            "imgu": imgu.reshape(RPC, 4 * W).astype(bf),
            "imgv": imgv.reshape(RPC + HALO, 4 * W).astype(bf),
            "lg": lg.reshape(RPC + HALO, 3 * W),
            "ds": ds,
            "tgt": tgt,
            "dsmall": dsmall.reshape(RPC, SPAN * W).astype(bf),
        })
    return in_maps


def _dens(destination_map):
    """Exact denominators per offset via integral image (f64)."""
    d = np.asarray(destination_map[:, 0]).astype(np.float64).sum(axis=0)
    ii = np.zeros((H + 1, W + 1))
    ii[1:, 1:] = d.cumsum(0).cumsum(1)

    def rect(r0, r1, c0, c1):
        return ii[r1, c1] - ii[r0, c1] - ii[r1, c0] + ii[r0, c0]

    den_pos = np.zeros(NPAIR)
    den_neg = np.zeros(NPAIR)
    for k, (dx, dy) in enumerate(PAIRS):
        if dy > 0:
            den_pos[k] = rect(0, H - dx, 0, W - dy)
            den_neg[k] = rect(dx, H, dy, W)
        else:
            den_pos[k] = rect(0, H - dx, -dy, W)
            den_neg[k] = rect(dx, H, 0, W + dy)
    return den_pos, den_neg


def _run(inputs, trace=False):
    from concourse.bass_utils import run_bass_kernel_spmd
    nc = _get_nc()
    in_maps = _make_inputs(inputs["logit"], inputs["target"], inputs["image"],
                           inputs["depth"], inputs["destination_map"])
    res = run_bass_kernel_spmd(nc, in_maps, core_ids=list(range(NCORES)),
                               trace=trace)
    outs = np.stack([np.asarray(res.results[i]["out"], np.float64)
                     for i in range(NCORES)])  # (8, 128, 1024)
    outs = outs[:, :, 0:ACC_W] + outs[:, :, ACC_W:2 * ACC_W]
    return outs, res


def _post(outs, destination_map):
    tot = outs.sum(axis=(0, 1))  # (512,)
    num_pos = tot[:NPAIR] * HOST_NUM_SCALE
    num_neg = tot[NPAIR:2 * NPAIR] * HOST_NUM_SCALE
    den_pos, den_neg = _dens(destination_map)
    e_sum = (num_pos / den_pos).sum() + (num_neg / den_neg).sum()
    K2 = (2 * SPAN + 1) ** 2
    l_gcrf = e_sum / K2

    n = B * H * W
    sum_lds = tot[COL_CE_LDS]
    sum_l = tot[COL_CE_L]
    l1 = sum_lds / n
    l2 = (sum_l - sum_lds) / n
    count = float(np.asarray(destination_map, np.float64).mean())
    ce = l1 * (1.0 - count) + l2 * count
    return np.float32(ce), np.float32(l_gcrf)


def kernel(logit, target, image, depth, destination_map, source_map):
    inputs = dict(logit=logit, target=target, image=image, depth=depth,
                  destination_map=destination_map)
    outs, _ = _run(inputs)
    return _post(outs, destination_map)

